# revision 12
# baseline (speedup 1.0000x reference)
"""Trainium2 Bass kernel for nn_CalculateVector (SAD block-match + spiral-tie argmin).

Contract: kernel(**inputs) takes the FULL inputs
  windows   [1,135,240,82,16] f32   (integer-valued, 0..255)
  templates [1,135,240, 1,16] f32
  search_range = 4
and returns the full outputs matching reference():
  (vector[1,135,240,2] f16, min_templates[1,135,240,1,16] f32,
   input_mv_mask[1,135,240,1] bool, min_cost_volume[1,135,240,1] int32)

Strategy: pure data parallel over the 32400 pixels, 4050 pixels per core
(padded to 4096 = 32 tiles of 128 partitions), SPMD on 8 NeuronCores.
Per pixel tile:
  cost[c] = sum_k |w[c,k] - t[k]|   (DVE subtract -> ACT |.| -> DVE grouped reduce)
All costs are small exact integers in f32, so the spiral tie-broken argmin is
computed exactly as argmin of key = cost*128 + tie[c] (tie = spiral rank - 1
for the 81 block-match channels, 127 for the input-MV channel).

The device returns, per pixel: the block-match argmin index, the input-MV
mask, and the min cost.  The tiny index-driven output assembly (the 81-entry
motion-vector LUT and the 64-byte min_templates row pick) happens on host.

Raw Bass (not Tile): this toolchain's walrus accepts at most ONE semaphore
wait per instruction, so all cross-engine synchronization is explicit
standalone wait_ge instructions; CoreSim's race detector also requires
explicit semaphore edges even for same-engine RAW pairs.
"""

import os
import sys

import numpy as np

for _p in ("/opt/trn_rl_repo",):
    if _p not in sys.path and os.path.isdir(_p):
        sys.path.insert(0, _p)

import concourse.bass as bass
import concourse.mybir as mybir
from concourse.bass_utils import run_bass_kernel_spmd

dt = mybir.dt
Alu = mybir.AluOpType
Act = mybir.ActivationFunctionType
Axis = mybir.AxisListType

B, H, W = 1, 135, 240
N = B * H * W            # 32400 pixels
NCORES = 8
PC = N // NCORES         # 4050 pixels per core
P = 128                  # partitions
NT = 32                  # pixel tiles per core
PAD = P * NT             # 4096 padded pixels per core
SR = 4
C = (2 * SR + 1) ** 2 + 1  # 82 channels (81 block-match + 1 input MV)
CBM = C - 1              # 81
K = 16
CK = C * K               # 1312

_F32 = dt.float32
_BF16 = dt.bfloat16

NWBUF = 3                # wt load buffering depth
NDBUF = 2                # d / ad buffering depth


def _spiral_pattern(sz: int) -> np.ndarray:
    pattern = np.zeros((2 * sz + 1, 2 * sz + 1), dtype=np.float32)
    radius = sz
    n = 0
    for dy in range(radius + 1):
        for direction in range(4):
            dx_start = -dy
            limit = 1 if direction == 0 and dy == 0 else dy
            for dx in range(dx_start, limit):
                if direction == 0:
                    wx_b, wy_b = radius + dx, radius + dy
                elif direction == 1:
                    wx_b, wy_b = radius + dy, radius - dx
                elif direction == 2:
                    wx_b, wy_b = radius - dx, radius - dy
                else:
                    wx_b, wy_b = radius - dy, radius + dx
                n += 1
                pattern[wy_b, wx_b] = n
    return pattern


def _consts():
    spiral = _spiral_pattern(SR).reshape(CBM)  # ranks 1..81 per channel
    tie = np.empty((C,), np.float32)
    tie[:CBM] = spiral - 1.0      # 0..80, unique per block-match channel
    tie[CBM] = 127.0              # input-MV channel loses all cost ties
    tie_t = np.broadcast_to(tie, (P, C)).copy()
    iota_t = np.broadcast_to(np.arange(CBM, dtype=np.float32), (P, CBM)).copy()
    return tie_t, iota_t


_PROGRAM_CACHE: dict = {}


def _build_program():
    if "nc" in _PROGRAM_CACHE:
        return _PROGRAM_CACHE["nc"]

    nc = bass.Bass(trn_type="TRN2")
    win = nc.dram_tensor("win", [PAD, CK], _F32, kind="ExternalInput").ap()
    tmpl = nc.dram_tensor("tmpl", [PAD, K], _F32, kind="ExternalInput").ap()
    tie_d = nc.dram_tensor("tie", [P, C], _F32, kind="ExternalInput").ap()
    iota_d = nc.dram_tensor("iota81", [P, CBM], _F32, kind="ExternalInput").ap()

    o_idx = nc.dram_tensor("o_idx", [P, NT], _F32, kind="ExternalOutput").ap()
    o_mask = nc.dram_tensor("o_mask", [P, NT], _F32, kind="ExternalOutput").ap()
    o_cost = nc.dram_tensor("o_cost", [P, NT], _F32, kind="ExternalOutput").ap()

    from contextlib import ExitStack

    with ExitStack() as ctx:
        e = ctx.enter_context

        # SBUF tensors
        wt = [e(nc.sbuf_tensor(f"wt{i}", [P, CK], _F32)) for i in range(NWBUF)]
        dd = [e(nc.sbuf_tensor(f"dd{i}", [P, CK], _BF16)) for i in range(NDBUF)]
        ad = [e(nc.sbuf_tensor(f"ad{i}", [P, CK], _BF16)) for i in range(NDBUF)]
        tmpl_sb = e(nc.sbuf_tensor([P, NT * K], _F32))
        tie_sb = e(nc.sbuf_tensor([P, C], _F32))
        iota_sb = e(nc.sbuf_tensor([P, CBM], _F32))
        cost_sb = e(nc.sbuf_tensor([P, NT * C], _F32))
        key_sb = e(nc.sbuf_tensor([P, NT * C], _F32))
        m_sb = e(nc.sbuf_tensor([P, NT * CBM], _F32))
        mi_sb = e(nc.sbuf_tensor([P, NT * CBM], _F32))
        kminb = e(nc.sbuf_tensor([P, NT], _F32))
        idx_sb = e(nc.sbuf_tensor([P, NT], _F32))
        mcost = e(nc.sbuf_tensor([P, NT], _F32))
        mask_sb = e(nc.sbuf_tensor([P, NT], _F32))

        # semaphores
        s_c = e(nc.semaphore())    # const loads
        s_w = [e(nc.semaphore(f"s_w{i}")) for i in range(NWBUF)]  # wt loads
        s_d = e(nc.semaphore())    # subtract done
        s_a = e(nc.semaphore())    # abs done
        s_r = e(nc.semaphore())    # reduce done
        s_x = e(nc.semaphore())    # batched results ready
        s_v = e(nc.semaphore())    # DVE batched chain
        s_st = e(nc.semaphore())   # stores done

        block = e(nc.Block())

        def c3(ap2d, inner):  # [P, A*inner] -> [P, A, inner]
            return ap2d.rearrange("p (a b) -> p a b", b=inner)

        @block.sync
        def _(sync):
            # const loads
            sync.dma_start(
                out=tmpl_sb[:].rearrange("p (t k) -> p t k", k=K),
                in_=tmpl.rearrange("(t p) k -> p t k", p=P),
            ).then_inc(s_c, 16)
            sync.dma_start(out=tie_sb[:], in_=tie_d).then_inc(s_c, 16)
            sync.dma_start(out=iota_sb[:], in_=iota_d).then_inc(s_c, 16)
            # window tile loads
            for i in range(NT):
                if i >= NWBUF:
                    # wt[i % NWBUF] last read by subtract (i - NWBUF)
                    sync.wait_ge(s_d, i - NWBUF + 1)
                sync.dma_start(
                    out=wt[i % NWBUF][:], in_=win[i * P:(i + 1) * P, :]
                ).then_inc(s_w[i % NWBUF], 16)
            # stores
            sync.wait_ge(s_x, 1)
            sync.dma_start(out=o_idx, in_=idx_sb[:]).then_inc(s_st, 16)
            sync.dma_start(out=o_mask, in_=mask_sb[:]).then_inc(s_st, 16)
            sync.dma_start(out=o_cost, in_=mcost[:]).then_inc(s_st, 16)
            sync.wait_ge(s_st, 48)

        @block.scalar
        def _(scalar):
            for i in range(NT):
                scalar.wait_ge(s_d, i + 1)
                if i >= NDBUF:
                    # ad[i % NDBUF] last read by reduce (i - NDBUF)
                    scalar.wait_ge(s_r, i - NDBUF + 1)
                nc.scalar.activation(
                    out=ad[i % NDBUF][:], in_=dd[i % NDBUF][:], func=Act.Abs
                ).then_inc(s_a, 1)

        @block.vector
        def _(vector):
            vector.wait_ge(s_c, 48)
            for i in range(NT):
                # subtract i
                vector.wait_ge(s_w[i % NWBUF], 16 * (i // NWBUF + 1))
                tb = (
                    tmpl_sb[:, i * K:(i + 1) * K]
                    .rearrange("p (o k) -> p o k", o=1)
                    .to_broadcast([P, C, K])
                )
                nc.vector.tensor_tensor(
                    out=c3(dd[i % NDBUF][:], K), in0=c3(wt[i % NWBUF][:], K),
                    in1=tb, op=Alu.subtract,
                ).then_inc(s_d, 1)
                # reduce i-1 (after ACT abs i-1)
                if i >= 1:
                    j = i - 1
                    vector.wait_ge(s_a, j + 1)
                    nc.vector.tensor_reduce(
                        out=cost_sb[:, j * C:(j + 1) * C],
                        in_=c3(ad[j % NDBUF][:], K), axis=Axis.X, op=Alu.add,
                    ).then_inc(s_r, 1)
            j = NT - 1
            vector.wait_ge(s_a, j + 1)
            nc.vector.tensor_reduce(
                out=cost_sb[:, j * C:(j + 1) * C],
                in_=c3(ad[j % NDBUF][:], K), axis=Axis.X, op=Alu.add,
            ).then_inc(s_r, 1)

            # ---- batched per-pixel machinery (all DVE; the race detector
            # needs explicit sem edges even same-engine, so chain via s_v) --
            vector.wait_ge(s_r, NT)
            cost3 = c3(cost_sb[:], C)
            key3 = c3(key_sb[:], C)
            tie_b = tie_sb[:].rearrange("p (o c) -> p o c", o=1).to_broadcast([P, NT, C])
            nc.vector.scalar_tensor_tensor(
                out=key3, in0=cost3, scalar=128.0, in1=tie_b,
                op0=Alu.mult, op1=Alu.add,
            ).then_inc(s_v, 1)
            nv = 1

            def step(emit):
                nonlocal nv
                vector.wait_ge(s_v, nv)
                emit().then_inc(s_v, 1)
                nv += 1

            m3 = c3(m_sb[:], CBM)
            mi3 = c3(mi_sb[:], CBM)
            kminb_b = kminb[:].rearrange("p (t o) -> p t o", o=1).to_broadcast([P, NT, CBM])
            iota_b = iota_sb[:].rearrange("p (o c) -> p o c", o=1).to_broadcast([P, NT, CBM])
            key81 = key3[:, :, CBM:C].rearrange("p t o -> p (t o)")
            step(lambda: nc.vector.tensor_reduce(
                out=kminb[:], in_=key3[:, :, 0:CBM], axis=Axis.X, op=Alu.min))
            step(lambda: nc.vector.tensor_tensor(
                out=m3, in0=key3[:, :, 0:CBM], in1=kminb_b, op=Alu.is_equal))
            step(lambda: nc.vector.tensor_tensor(out=mi3, in0=m3, in1=iota_b, op=Alu.mult))
            step(lambda: nc.vector.tensor_reduce(out=idx_sb[:], in_=mi3, axis=Axis.X, op=Alu.add))
            step(lambda: nc.vector.tensor_reduce(out=mcost[:], in_=cost3, axis=Axis.X, op=Alu.min))
            step(lambda: nc.vector.tensor_tensor(
                out=mask_sb[:], in0=key81, in1=kminb[:], op=Alu.is_lt))
            vector.wait_ge(s_v, nv)
            nc.vector.memset(kminb[:, 0:1], 0.0).then_inc(s_x, 1)

    _PROGRAM_CACHE["nc"] = nc
    return nc


def build_in_maps(windows: np.ndarray, templates: np.ndarray):
    wf = np.ascontiguousarray(windows, dtype=np.float32).reshape(N, CK)
    tf = np.ascontiguousarray(templates, dtype=np.float32).reshape(N, K)
    tie_t, iota_t = _consts()
    in_maps = []
    for c in range(NCORES):
        wslice = np.zeros((PAD, CK), np.float32)
        wslice[:PC] = wf[c * PC:(c + 1) * PC]
        tslice = np.zeros((PAD, K), np.float32)
        tslice[:PC] = tf[c * PC:(c + 1) * PC]
        in_maps.append({
            "win": wslice,
            "tmpl": tslice,
            "tie": tie_t,
            "iota81": iota_t,
        })
    return in_maps


def _unscramble(per_core: list[np.ndarray]) -> np.ndarray:
    """[P, NT, ...] per core -> [N, ...] pixel-ordered."""
    arr = np.stack(per_core)  # [NCORES, P, NT, ...]
    tail = arr.shape[3:]
    arr = np.moveaxis(arr, 2, 1)          # [NCORES, NT, P, ...]
    arr = arr.reshape(NCORES, PAD, *tail)[:, :PC]
    return arr.reshape(N, *tail)


def assemble_outputs(results: list[dict], windows: np.ndarray):
    idxv = np.rint(_unscramble([r["o_idx"] for r in results])).astype(np.int64)
    maskv = _unscramble([r["o_mask"] for r in results]) > 0.5
    costv = np.rint(_unscramble([r["o_cost"] for r in results])).astype(np.int32)

    vy = (4 - idxv // 9).astype(np.float16)
    vx = (4 - idxv % 9).astype(np.float16)
    vector = np.stack([vy, vx], axis=-1).reshape(B, H, W, 2)

    # min_templates: 64-byte row pick per pixel, driven by the device indices
    midx = np.where(maskv, CBM, idxv)
    wfull = np.ascontiguousarray(windows, np.float32).reshape(N, C, K)
    min_templates = np.take_along_axis(
        wfull, midx[:, None, None].astype(np.int64), axis=1
    ).reshape(B, H, W, 1, K)

    input_mv_mask = maskv.reshape(B, H, W, 1)
    min_cost_volume = costv.reshape(B, H, W, 1)
    return vector, min_templates, input_mv_mask, min_cost_volume


LAST_EXEC_NS = None


def kernel(windows, templates, search_range):
    global LAST_EXEC_NS
    assert int(search_range) == SR, f"kernel hardcodes search_range={SR}"
    windows = np.asarray(windows)
    nc = _build_program()
    in_maps = build_in_maps(windows, np.asarray(templates))
    trace = bool(int(os.environ.get("KERNEL_TRACE", "0")))
    out = run_bass_kernel_spmd(nc, in_maps, list(range(NCORES)), trace=trace)
    LAST_EXEC_NS = out.exec_time_ns
    return assemble_outputs(out.results, windows)


# revision 16
# speedup vs baseline: 49.5924x; 49.5924x over previous
"""Trainium2 Bass kernel for nn_CalculateVector (SAD block-match + spiral-tie argmin).

Contract: kernel(**inputs) takes the FULL inputs
  windows   [1,135,240,82,16] f32   (integer-valued, 0..255)
  templates [1,135,240, 1,16] f32
  search_range = 4
and returns the full outputs matching reference():
  (vector[1,135,240,2] f16, min_templates[1,135,240,1,16] f32,
   input_mv_mask[1,135,240,1] bool, min_cost_volume[1,135,240,1] int32)

Strategy: pure data parallel over the 32400 pixels, 4050 pixels per core
(padded to 4096 = 32 tiles of 128 partitions), SPMD on 8 NeuronCores.
Per pixel tile:
  cost[c] = sum_k |w[c,k] - t[k]|   (DVE subtract -> ACT |.| -> DVE grouped reduce)
All costs are small exact integers in f32, so the spiral tie-broken argmin is
computed exactly as argmin of key = cost*128 + tie[c] (tie = spiral rank - 1
for the 81 block-match channels, 127 for the input-MV channel).

The device returns, per pixel: the block-match argmin index, the input-MV
mask, and the min cost.  The tiny index-driven output assembly (the 81-entry
motion-vector LUT and the 64-byte min_templates row pick) happens on host.

Raw Bass (not Tile): this toolchain's walrus accepts at most ONE semaphore
wait per instruction, so all cross-engine synchronization is explicit
standalone wait_ge instructions; CoreSim's race detector also requires
explicit semaphore edges even for same-engine RAW pairs.
"""

import os
import sys

import numpy as np

for _p in ("/opt/trn_rl_repo",):
    if _p not in sys.path and os.path.isdir(_p):
        sys.path.insert(0, _p)

import concourse.bass as bass
import concourse.mybir as mybir
from concourse.bass_utils import run_bass_kernel_spmd

dt = mybir.dt
Alu = mybir.AluOpType
Act = mybir.ActivationFunctionType
Axis = mybir.AxisListType

B, H, W = 1, 135, 240
N = B * H * W            # 32400 pixels
NCORES = 8
PC = N // NCORES         # 4050 pixels per core
P = 128                  # partitions
NT = 32                  # pixel tiles per core
PAD = P * NT             # 4096 padded pixels per core
SR = 4
C = (2 * SR + 1) ** 2 + 1  # 82 channels (81 block-match + 1 input MV)
CBM = C - 1              # 81
K = 16
CK = C * K               # 1312

_F32 = dt.float32
_BF16 = dt.bfloat16

NWBUF = 3                # wt load buffering depth
NDBUF = 2                # d / ad buffering depth


def _spiral_pattern(sz: int) -> np.ndarray:
    pattern = np.zeros((2 * sz + 1, 2 * sz + 1), dtype=np.float32)
    radius = sz
    n = 0
    for dy in range(radius + 1):
        for direction in range(4):
            dx_start = -dy
            limit = 1 if direction == 0 and dy == 0 else dy
            for dx in range(dx_start, limit):
                if direction == 0:
                    wx_b, wy_b = radius + dx, radius + dy
                elif direction == 1:
                    wx_b, wy_b = radius + dy, radius - dx
                elif direction == 2:
                    wx_b, wy_b = radius - dx, radius - dy
                else:
                    wx_b, wy_b = radius - dy, radius + dx
                n += 1
                pattern[wy_b, wx_b] = n
    return pattern


def _consts():
    spiral = _spiral_pattern(SR).reshape(CBM)  # ranks 1..81 per channel
    tie = np.empty((C,), np.float32)
    tie[:CBM] = spiral - 1.0      # 0..80, unique per block-match channel
    tie[CBM] = 127.0              # input-MV channel loses all cost ties
    tie_t = np.broadcast_to(tie, (P, C)).copy()
    iota_t = np.broadcast_to(np.arange(CBM, dtype=np.float32), (P, CBM)).copy()
    return tie_t, iota_t


_PROGRAM_CACHE: dict = {}


def _build_program(repeat: int = 1):
    """Build the SPMD program. repeat>1 re-runs the whole pipeline that many
    times inside one NEFF (same data) — used only for wall-clock slope timing."""
    key = ("nc", repeat)
    if key in _PROGRAM_CACHE:
        return _PROGRAM_CACHE[key]

    nc = bass.Bass(trn_type="TRN2")
    win = nc.dram_tensor("win", [PAD, CK], _F32, kind="ExternalInput").ap()
    tmpl = nc.dram_tensor("tmpl", [PAD, K], _F32, kind="ExternalInput").ap()
    tie_d = nc.dram_tensor("tie", [P, C], _F32, kind="ExternalInput").ap()
    iota_d = nc.dram_tensor("iota81", [P, CBM], _F32, kind="ExternalInput").ap()

    o_kmin = nc.dram_tensor("o_kmin", [P, NT], _F32, kind="ExternalOutput").ap()
    o_c81 = nc.dram_tensor("o_c81", [P, NT], _F32, kind="ExternalOutput").ap()

    from contextlib import ExitStack

    NTG = NT * repeat  # total tile iterations across rounds

    with ExitStack() as ctx:
        e = ctx.enter_context

        # SBUF tensors
        wt = [e(nc.sbuf_tensor(f"wt{i}", [P, CK], _F32)) for i in range(NWBUF)]
        dd = [e(nc.sbuf_tensor(f"dd{i}", [P, CK], _BF16)) for i in range(NDBUF)]
        ad = [e(nc.sbuf_tensor(f"ad{i}", [P, CK], _BF16)) for i in range(NDBUF)]
        tmpl_sb = e(nc.sbuf_tensor([P, NT * K], _F32))
        tie_sb = e(nc.sbuf_tensor([P, C], _F32))
        iota_sb = e(nc.sbuf_tensor([P, CBM], _F32))
        uu = [e(nc.sbuf_tensor(f"uu{i}", [P, C * 8], _F32)) for i in range(NDBUF)]
        cost_sb = e(nc.sbuf_tensor([P, NT * C], _F32))
        key_sb = e(nc.sbuf_tensor([P, NT * CBM], _F32))
        kminb = e(nc.sbuf_tensor([P, NT], _F32))
        c81_sb2 = e(nc.sbuf_tensor([P, NT], _F32))

        # semaphores
        s_c = e(nc.semaphore())    # const loads
        s_w = [e(nc.semaphore(f"s_w{i}")) for i in range(NWBUF)]  # wt loads
        s_d = e(nc.semaphore())    # subtract done
        s_a = e(nc.semaphore())    # abs done
        s_u = e(nc.semaphore())    # gpsimd halving done
        s_r = e(nc.semaphore())    # reduce done
        s_x = e(nc.semaphore())    # batched results ready
        s_v = e(nc.semaphore())    # DVE batched chain
        s_st = e(nc.semaphore())   # stores done

        block = e(nc.Block())

        def c3(ap2d, inner):  # [P, A*inner] -> [P, A, inner]
            return ap2d.rearrange("p (a b) -> p a b", b=inner)

        NV_PER_ROUND = 7

        @block.sync
        def _(sync):
            # const loads
            sync.dma_start(
                out=tmpl_sb[:].rearrange("p (t k) -> p t k", k=K),
                in_=tmpl.rearrange("(t p) k -> p t k", p=P),
            ).then_inc(s_c, 16)
            sync.dma_start(out=tie_sb[:], in_=tie_d).then_inc(s_c, 16)
            sync.dma_start(out=iota_sb[:], in_=iota_d).then_inc(s_c, 16)
            # window tile loads (g = global iteration index)
            for g in range(NTG):
                i = g % NT
                if g >= NWBUF:
                    # wt[g % NWBUF] last read by subtract (g - NWBUF)
                    sync.wait_ge(s_d, g - NWBUF + 1)
                sync.dma_start(
                    out=wt[g % NWBUF][:], in_=win[i * P:(i + 1) * P, :]
                ).then_inc(s_w[g % NWBUF], 16)
            # stores (final round only)
            sync.wait_ge(s_x, repeat)
            sync.dma_start(out=o_kmin, in_=kminb[:]).then_inc(s_st, 16)
            sync.dma_start(out=o_c81, in_=c81_sb2[:]).then_inc(s_st, 16)
            sync.wait_ge(s_st, 32)

        @block.scalar
        def _(scalar):
            for g in range(NTG):
                scalar.wait_ge(s_d, g + 1)
                if g >= NDBUF:
                    # ad[g % NDBUF] last read by gpsimd halving (g - NDBUF)
                    scalar.wait_ge(s_u, g - NDBUF + 1)
                nc.scalar.activation(
                    out=ad[g % NDBUF][:], in_=dd[g % NDBUF][:], func=Act.Abs
                ).then_inc(s_a, 1)

        @block.gpsimd
        def _(gpsimd):
            for g in range(NTG):
                gpsimd.wait_ge(s_a, g + 1)
                if g >= NDBUF:
                    # uu[g % NDBUF] last read by reduce2 (g - NDBUF)
                    gpsimd.wait_ge(s_r, g - NDBUF + 1)
                adp = ad[g % NDBUF][:]
                lo = adp.rearrange("p (c two k8) -> p c two k8", two=2, k8=8)[:, :, 0, :]
                hi = adp.rearrange("p (c two k8) -> p c two k8", two=2, k8=8)[:, :, 1, :]
                nc.gpsimd.tensor_tensor(
                    out=c3(uu[g % NDBUF][:], 8), in0=lo, in1=hi, op=Alu.add,
                ).then_inc(s_u, 1)

        @block.vector
        def _(vector):
            vector.wait_ge(s_c, 48)
            nv = 0

            def step(emit):
                nonlocal nv
                vector.wait_ge(s_v, nv)
                emit().then_inc(s_v, 1)
                nv += 1

            cost3 = c3(cost_sb[:], C)
            key3 = c3(key_sb[:], CBM)
            tie_b = (
                tie_sb[:, 0:CBM]
                .rearrange("p (o c) -> p o c", o=1)
                .to_broadcast([P, NT, CBM])
            )

            for rnd in range(repeat):
                if rnd >= 1:
                    # cross-round WAR edges for the race detector
                    vector.wait_ge(s_v, nv)
                for i in range(NT):
                    g = rnd * NT + i
                    # subtract g
                    vector.wait_ge(s_w[g % NWBUF], 16 * (g // NWBUF + 1))
                    if g >= NDBUF:
                        # dd[g % NDBUF] last read by abs (g - NDBUF)
                        vector.wait_ge(s_a, g - NDBUF + 1)
                    tb = (
                        tmpl_sb[:, i * K:(i + 1) * K]
                        .rearrange("p (o k) -> p o k", o=1)
                        .to_broadcast([P, C, K])
                    )
                    nc.vector.tensor_tensor(
                        out=c3(dd[g % NDBUF][:], K), in0=c3(wt[g % NWBUF][:], K),
                        in1=tb, op=Alu.subtract,
                    ).then_inc(s_d, 1)
                    # reduce g-2 (after gpsimd halving g-2)
                    if g >= 2:
                        h = g - 2
                        vector.wait_ge(s_u, h + 1)
                        nc.vector.tensor_reduce(
                            out=cost_sb[:, (h % NT) * C:((h % NT) + 1) * C],
                            in_=c3(uu[h % NDBUF][:], 8), axis=Axis.X, op=Alu.add,
                        ).then_inc(s_r, 1)
                for h in range((rnd + 1) * NT - 2, (rnd + 1) * NT):
                    vector.wait_ge(s_u, h + 1)
                    nc.vector.tensor_reduce(
                        out=cost_sb[:, (h % NT) * C:((h % NT) + 1) * C],
                        in_=c3(uu[h % NDBUF][:], 8), axis=Axis.X, op=Alu.add,
                    ).then_inc(s_r, 1)

                # ---- slim batched tail: key over BM channels, min, cost81 copy
                vector.wait_ge(s_r, (rnd + 1) * NT)
                step(lambda: nc.vector.scalar_tensor_tensor(
                    out=key3, in0=cost3[:, :, 0:CBM], scalar=128.0, in1=tie_b,
                    op0=Alu.mult, op1=Alu.add))
                step(lambda: nc.vector.tensor_reduce(
                    out=kminb[:], in_=key3, axis=Axis.X, op=Alu.min))
                step(lambda: nc.vector.tensor_copy(
                    out=c81_sb2[:],
                    in_=cost3[:, :, CBM:C].rearrange("p t o -> p (t o)")))
                vector.wait_ge(s_v, nv)
                nc.vector.memset(key_sb[:, 0:1], 0.0).then_inc(s_x, 1)

    _PROGRAM_CACHE[key] = nc
    return nc


def build_in_maps(windows: np.ndarray, templates: np.ndarray):
    wf = np.ascontiguousarray(windows, dtype=np.float32).reshape(N, CK)
    tf = np.ascontiguousarray(templates, dtype=np.float32).reshape(N, K)
    tie_t, iota_t = _consts()
    in_maps = []
    for c in range(NCORES):
        wslice = np.zeros((PAD, CK), np.float32)
        wslice[:PC] = wf[c * PC:(c + 1) * PC]
        tslice = np.zeros((PAD, K), np.float32)
        tslice[:PC] = tf[c * PC:(c + 1) * PC]
        in_maps.append({
            "win": wslice,
            "tmpl": tslice,
            "tie": tie_t,
            "iota81": iota_t,
        })
    return in_maps


def _unscramble(per_core: list[np.ndarray]) -> np.ndarray:
    """[P, NT, ...] per core -> [N, ...] pixel-ordered."""
    arr = np.stack(per_core)  # [NCORES, P, NT, ...]
    tail = arr.shape[3:]
    arr = np.moveaxis(arr, 2, 1)          # [NCORES, NT, P, ...]
    arr = arr.reshape(NCORES, PAD, *tail)[:, :PC]
    return arr.reshape(N, *tail)


_SPIRAL_INV = None


def _spiral_inv():
    global _SPIRAL_INV
    if _SPIRAL_INV is None:
        spiral = _spiral_pattern(SR).reshape(CBM).astype(np.int64)  # ranks 1..81
        inv = np.zeros(CBM, np.int64)
        inv[spiral - 1] = np.arange(CBM)
        _SPIRAL_INV = inv
    return _SPIRAL_INV


def assemble_outputs(results: list[dict], windows: np.ndarray):
    kminv = np.rint(_unscramble([r["o_kmin"] for r in results])).astype(np.int64)
    c81v = np.rint(_unscramble([r["o_c81"] for r in results])).astype(np.int64)
    cost_bm = kminv >> 7           # kmin = cost*128 + (spiral_rank - 1)
    rank_w = kminv & 127
    idxv = _spiral_inv()[rank_w]
    maskv = c81v < cost_bm
    costv = np.minimum(c81v, cost_bm).astype(np.int32)

    vy = (4 - idxv // 9).astype(np.float16)
    vx = (4 - idxv % 9).astype(np.float16)
    vector = np.stack([vy, vx], axis=-1).reshape(B, H, W, 2)

    # min_templates: 64-byte row pick per pixel, driven by the device indices
    midx = np.where(maskv, CBM, idxv)
    wfull = np.ascontiguousarray(windows, np.float32).reshape(N, C, K)
    min_templates = np.take_along_axis(
        wfull, midx[:, None, None].astype(np.int64), axis=1
    ).reshape(B, H, W, 1, K)

    input_mv_mask = maskv.reshape(B, H, W, 1)
    min_cost_volume = costv.reshape(B, H, W, 1)
    return vector, min_templates, input_mv_mask, min_cost_volume


LAST_EXEC_NS = None


def kernel(windows, templates, search_range):
    global LAST_EXEC_NS
    assert int(search_range) == SR, f"kernel hardcodes search_range={SR}"
    windows = np.asarray(windows)
    nc = _build_program()
    in_maps = build_in_maps(windows, np.asarray(templates))
    trace = bool(int(os.environ.get("KERNEL_TRACE", "0")))
    out = run_bass_kernel_spmd(nc, in_maps, list(range(NCORES)), trace=trace)
    LAST_EXEC_NS = out.exec_time_ns
    return assemble_outputs(out.results, windows)
